# revision 21
# baseline (speedup 1.0000x reference)
"""GAT attention layer (nn_AttentionLayer) on 8 Trainium2 NeuronCores.

Row-sharded outputs: core c owns output rows I_c = [c*N/8, (c+1)*N/8).
Host-side staging (same values, layout/precision choices only):
    adjP[q] for q<8  = adj-block(q).T * 40            fp16 {0,40}
    adjP[q] for q>=8 = (adj-block(q)-adj-block(q-8)).T * 40  {-40,0,40}
    featP = features.T tiled [NXC, 128, nk, jxc]      fp16 (replicated)
    featT_loc = features[I_c].T                       fp16
Device pipeline per 512-row j-quad (j on partitions, local i free):
    q < 8 : slot <- DMA(adjP[q]);  slot += s1_bcast   (DVE TT 2x)
    q >= 8: slot += DMA-accum(adjP[q])                (SWDGE compute DMA;
            slot already holds adj[q-8]*40 + s1, delta restores adj[q]+s1)
    y    = prelu(slot + (s2_j-40), 0.2) -> work       (ACT, bias per tile;
                                         some tiles on DVE TS+STT instead)
    bits = int16(y*A + B)                (DVE TS 4x; Schraudolph exp:
                                          bitcast fp16 ~= exp(y))
    psum[c,i] += hs[j,c].T @ bits.as_fp16[j,i]        (hs as PE weights)
    out = elu(num/den) after a PE transpose of the [67, rl] accumulator.
Masked entries: x ~= s1+s2-40 -> y ~= 0.2x -> exp(y) ~ e^-8, negligible
vs row mass; no explicit mask multiply or -inf needed.
"""

import os
import sys

for _p in ("/opt/trn_rl_repo",):
    if os.path.isdir(_p) and _p not in sys.path:
        sys.path.append(_p)

import numpy as np

import concourse.bass as bass
import concourse.bacc as bacc
import concourse.mybir as mybir
import concourse.tile as tile
from concourse import bass_utils

N, D, F = 8192, 256, 64
NCORES = 8
RL = N // NCORES
BIG = 40.0
ALPHA = 0.2
# Schraudolph fp16 exp: bitcast_f16(int16(A*y + B)) ~= e^y, A = 2^10/ln2,
# B = 15360 - 61 (61 centers the mantissa-linearization error).
SCH_A = 1477.3196
SCH_B = 15299.0
RING = 8                  # adj slot ring; quads q>=RING are delta-encoded
DVE_LEAKY = 0             # j-tiles whose leaky runs on DVE instead of ACT
USE_DMA_ACCUM = False     # DRAM->SBUF accum during DMA (SWDGE compute)

f32 = mybir.dt.float32
fp16 = mybir.dt.float16
i16 = mybir.dt.int16
Alu = mybir.AluOpType
Act = mybir.ActivationFunctionType

LAST_RESULTS = None
_CACHE = {}


def _kernel_body(tc, out_d, featP_d, featTl_d, adjP_d, W_d, a_d, idn_d, n=N, rl=RL):
    nc = tc.nc
    nit = rl // 128           # local i-tiles
    njt = n // 128            # global j-tiles
    nk = D // 128             # d contraction tiles
    QT = 4                    # j-tiles per chain quad
    NQ = njt // QT
    HC = F + 3                # hs cols: h(64) | s1 | s2 | ones
    NXC = 4                   # X^T streamed in chunks along j
    jxc = n // NXC

    with (
        tc.tile_pool(name="sbP", bufs=1) as sbP,
        tc.tile_pool(name="sbS", bufs=2) as sbS,
        tc.tile_pool(name="sbA", bufs=RING) as sbA,
        tc.tile_pool(name="sbW", bufs=3) as sbW,
        tc.tile_pool(name="sbL", bufs=3) as sbL,
        tc.tile_pool(name="sbE", bufs=4) as sbE,
        tc.tile_pool(name="pp", bufs=6, space="PSUM") as pp,
        tc.tile_pool(name="pacc", bufs=1, space="PSUM") as pacc,
    ):
        aq = [
            sbA.tile([128, QT, rl], fp16, tag="aq", name=f"aq{q}") for q in range(NQ)
        ]

        # ---- plain adj loads for the first RING quads: no deps, start t=0 -
        for q0 in range(min(RING, NQ)):
            nc.gpsimd.dma_start(aq[q0][:], adjP_d[q0])

        # ---- prologue feeds on the sync (HWDGE) queue ---------------------
        arow = sbP.tile([1, 2 * F], f32)
        nc.sync.dma_start(arow[:], a_d.rearrange("f o -> o f"))
        wsb = sbP.tile([128, nk, F], f32)
        nc.sync.dma_start(wsb[:], W_d.rearrange("(k p) f -> p k f", p=128))
        idn = sbP.tile([128, 128], f32)
        nc.sync.dma_start(idn[:], idn_d)
        xTl = sbP.tile([128, nk, rl], fp16)
        nc.sync.dma_start(xTl[:], featTl_d.rearrange("(k p) i -> p k i", p=128))

        xTf = [
            sbS.tile([128, nk, jxc], fp16, tag="xTf", name=f"xTf{i}")
            for i in range(NXC)
        ]
        for i in range(NXC):
            nc.sync.dma_start(xTf[i][:], featP_d[i])

        # ---- constants / rhs16 --------------------------------------------
        onesf = sbP.tile([1, 128], f32)
        nc.vector.memset(onesf[:], 1.0)
        ab = sbP.tile([128, 2 * F], f32)
        psab = pp.tile([128, 2 * F], f32, tag="big", name="psab")
        nc.tensor.matmul(psab[:], onesf[:], arow[:])
        nc.vector.tensor_copy(ab[:], psab[:])
        wa = sbP.tile([128, nk, 2], f32)
        scr = sbP.tile([128, F], f32)
        for k in range(nk):
            nc.vector.scalar_tensor_tensor(
                scr[:], wsb[:, k, :], 1.0, ab[:, :F], Alu.mult, Alu.mult,
                accum_out=wa[:, k, 0:1],
            )
            nc.vector.scalar_tensor_tensor(
                scr[:], wsb[:, k, :], 1.0, ab[:, F:], Alu.mult, Alu.mult,
                accum_out=wa[:, k, 1:2],
            )
        rhs16 = sbP.tile([128, nk, F + 2], fp16)
        for k in range(nk):
            nc.vector.tensor_copy(rhs16[:, k, :F], wsb[:, k, :])
            nc.vector.tensor_copy(rhs16[:, k, F : F + 2], wa[:, k, :])

        # ---- fast s1: [1, rl] row via wa1-as-weights matmul ---------------
        ones1 = sbP.tile([1, 128], fp16)
        nc.vector.memset(ones1[:], 1.0)
        s1row = sbP.tile([1, rl], fp16)
        for h in range(0, rl, 512):
            ps1 = pp.tile([1, 512], f32, tag="big", name=f"ps1_{h}")
            for k in range(nk):
                nc.tensor.matmul(
                    ps1[:], rhs16[:, k, F : F + 1], xTl[:, k, h : h + 512],
                    start=(k == 0), stop=(k == nk - 1),
                )
            nc.vector.tensor_copy(s1row[:, h : h + 512], ps1[:])
        s1b4 = sbP.tile([128, QT, rl], fp16)
        for h in range(0, rl, 512):
            psb = pp.tile([128, 512], f32, tag="big", name=f"psb{h}")
            nc.tensor.matmul(psb[:], ones1[:], s1row[:, h : h + 512])
            nc.vector.tensor_copy(s1b4[:, 0, h : h + 512], psb[:])
        for tt in range(1, QT):
            nc.vector.tensor_copy(s1b4[:, tt, :], s1b4[:, 0, :])

        # ---- [h|s1|s2|1] for all rows (batched psum, one copy per quad) ---
        hs_all = sbP.tile([128, njt, HC], fp16)
        nc.vector.memset(hs_all[:, :, F + 2 : F + 3], 1.0)
        for g in range(njt // QT):
            psh = pp.tile([128, QT, F + 2], f32, tag="big", name=f"psh{g}")
            for tt in range(QT):
                t = g * QT + tt
                cx, ct = divmod(t, jxc // 128)
                for k in range(nk):
                    nc.tensor.matmul(
                        psh[:, tt, :], xTf[cx][:, k, ct * 128 : (ct + 1) * 128],
                        rhs16[:, k, :],
                        start=(k == 0), stop=(k == nk - 1),
                    )
            nc.scalar.copy(hs_all[:, g * QT : (g + 1) * QT, : F + 2], psh[:])

        # s2 - 40, per-partition scalars per j-tile (fp32)
        s2mB = sbP.tile([128, njt], f32)

        # ---- attention chains over j-quads --------------------------------
        wk = [
            sbW.tile([128, QT, rl], fp16, tag="wk", name=f"wk{q}") for q in range(NQ)
        ]
        lt = [
            sbL.tile([128, QT, rl], i16, tag="lt", name=f"lt{q}") for q in range(NQ)
        ]
        # hs-as-weights accumulator: [HC, rl] += hs[:,t,:].T @ P[:, i]
        pso = pacc.tile([HC, rl], f32, name="pso")

        ndve = 0
        for q in range(NQ):
            nc.vector.tensor_scalar(
                s2mB[:, q * QT : (q + 1) * QT],
                hs_all[:, q * QT : (q + 1) * QT, F + 1],
                -BIG, None, Alu.add,
            )
            w = aq[q]
            y = wk[q]
            if q < RING or not USE_DMA_ACCUM:
                # x1 = adj*40 + s1 (in place; slot preserved afterwards)
                wflat = w[:].rearrange("p t i -> p (t i)")
                nc.vector.tensor_tensor(
                    wflat, wflat, s1b4[:].rearrange("p t i -> p (t i)"), Alu.add
                )
            for tt in range(QT):
                t = q * QT + tt
                if tt == QT - 1 and ndve < DVE_LEAKY:
                    ndve += 1
                    nc.vector.tensor_scalar(
                        y[:, tt, :], w[:, tt, :], s2mB[:, t : t + 1], None, Alu.add
                    )
                    nc.vector.scalar_tensor_tensor(
                        y[:, tt, :], y[:, tt, :], ALPHA, y[:, tt, :],
                        Alu.mult, Alu.max,
                    )
                else:
                    nc.scalar.activation(
                        y[:, tt, :], w[:, tt, :], Act.Prelu,
                        bias=s2mB[:, t : t + 1], alpha=ALPHA,
                    )
            ltf = lt[q][:].rearrange("p t i -> p (t i)")
            nc.vector.tensor_scalar(
                ltf, y[:].rearrange("p t i -> p (t i)"), SCH_A, SCH_B,
                Alu.mult, Alu.add,
            )
            w16 = lt[q][:].bitcast(fp16)
            for tt in range(QT):
                t = q * QT + tt
                for hh in range(0, rl, 512):
                    nc.tensor.matmul(
                        pso[:, hh : hh + 512], hs_all[:, t, :],
                        w16[:, tt, hh : hh + 512],
                        start=(t == 0), stop=(t == njt - 1),
                    )
            # DMA for quad q+RING lands on this quad's slot
            if q + RING < NQ:
                if USE_DMA_ACCUM:
                    nc.gpsimd.dma_start(
                        aq[q + RING][:], adjP_d[q + RING], accum_op=Alu.add
                    )
                else:
                    nc.gpsimd.dma_start(aq[q + RING][:], adjP_d[q + RING])

        # ---- epilogue: PE-transpose pso chunks, divide, elu ---------------
        psof = sbE.tile([HC, rl], f32, tag="psof", bufs=1)
        nc.vector.tensor_copy(psof[:], pso[:])
        for it in range(nit):
            pst = pp.tile([128, HC], f32, tag="big", name=f"pst{it}")
            nc.tensor.transpose(
                pst[:, :], psof[:, it * 128 : (it + 1) * 128], idn[:HC, :HC]
            )
            rcp = sbE.tile([128, 1], f32, tag="rcp")
            nc.vector.reciprocal(rcp[:], pst[:, F + 2 : F + 3])
            o = sbE.tile([128, F], f32, tag="o")
            nc.vector.tensor_scalar_mul(o[:], pst[:, :F], rcp[:])
            q2 = sbE.tile([128, F], f32, tag="q2")
            nc.vector.tensor_scalar_min(q2[:], o[:], 0.0)
            e = sbE.tile([128, F], f32, tag="e")
            nc.scalar.activation(e[:], q2[:], Act.Exp)
            r = sbE.tile([128, F], f32, tag="r")
            nc.vector.tensor_scalar_max(r[:], o[:], 0.0)
            fin = sbE.tile([128, F], f32, tag="fin")
            nc.vector.scalar_tensor_tensor(
                fin[:], e[:], -1.0, r[:], Alu.add, Alu.add
            )
            nc.sync.dma_start(out_d[it * 128 : (it + 1) * 128, :], fin[:])


def _build(n=N, rl=RL, ncores=NCORES):
    key = (n, rl, ncores)
    if key in _CACHE:
        return _CACHE[key]
    nc = bacc.Bacc(
        "TRN2", target_bir_lowering=False, debug=False, num_devices=ncores
    )
    njt = n // 128
    NQ = njt // 4
    jxc = n // 4
    nk = D // 128
    featP = nc.dram_tensor("featP", [4, 128, nk, jxc], fp16, kind="ExternalInput").ap()
    featTl = nc.dram_tensor("featTl", [D, rl], fp16, kind="ExternalInput").ap()
    adjP = nc.dram_tensor("adjP", [NQ, 128, 4, rl], fp16, kind="ExternalInput").ap()
    W = nc.dram_tensor("W", [D, F], f32, kind="ExternalInput").ap()
    a = nc.dram_tensor("a", [2 * F, 1], f32, kind="ExternalInput").ap()
    idn = nc.dram_tensor("idn", [128, 128], f32, kind="ExternalInput").ap()
    out = nc.dram_tensor("out", [rl, F], f32, kind="ExternalOutput").ap()
    with tile.TileContext(nc) as tc:
        _kernel_body(tc, out, featP, featTl, adjP, W, a, idn, n=n, rl=rl)
    nc.compile()
    _CACHE[key] = nc
    return nc


def kernel(features, adj, W, a):
    global LAST_RESULTS
    features = np.ascontiguousarray(features, dtype=np.float32)
    adj = np.ascontiguousarray(adj, dtype=np.int32)
    W = np.ascontiguousarray(W, dtype=np.float32)
    a = np.ascontiguousarray(a, dtype=np.float32)

    n = adj.shape[0]
    rl = n // NCORES
    njt = n // 128
    NQ = njt // 4
    nk = D // 128
    jxc = n // 4
    nc = _build(n=n, rl=rl, ncores=NCORES)
    # featP[c, p, k, j] = features.T[(k*128+p), c*jxc + j]
    fT16 = features.T.astype(np.float16)                    # [D, n]
    featP = np.ascontiguousarray(
        fT16.reshape(nk, 128, 4, jxc).transpose(2, 1, 0, 3)
    )
    idn = np.eye(128, dtype=np.float32)
    in_maps = []
    for c in range(NCORES):
        adjT40 = adj[c * rl : (c + 1) * rl].T.astype(np.float16) * np.float16(BIG)
        # adjP[Q, p, t, i] = adjT40[Q*512 + t*128 + p, i]; delta-encode Q>=RING
        adjP = adjT40.reshape(NQ, 4, 128, rl).transpose(0, 2, 1, 3).copy()
        if USE_DMA_ACCUM:
            adjP[RING:] = adjP[RING:] - adjP[: NQ - RING]
        in_maps.append(
            {
                "featP": featP,
                "featTl": np.ascontiguousarray(
                    features[c * rl : (c + 1) * rl].T.astype(np.float16)
                ),
                "adjP": np.ascontiguousarray(adjP),
                "W": W,
                "a": a,
                "idn": idn,
            }
        )
    res = bass_utils.run_bass_kernel_spmd(nc, in_maps, core_ids=list(range(NCORES)))
    LAST_RESULTS = res
    return np.concatenate([res.results[c]["out"] for c in range(NCORES)], axis=0)


# revision 23
# speedup vs baseline: 1.1878x; 1.1878x over previous
"""GAT attention layer (nn_AttentionLayer) on 8 Trainium2 NeuronCores.

Row-sharded outputs: core c owns output rows I_c = [c*N/8, (c+1)*N/8).
Host-side staging (same values, layout/precision choices only):
    adjP[q] for q<8  = adj-block(q).T * 40            fp16 {0,40}
    adjP[q] for q>=8 = (adj-block(q)-adj-block(q-8)).T * 40  {-40,0,40}
    featP = features.T tiled [NXC, 128, nk, jxc]      fp16 (replicated)
    featT_loc = features[I_c].T                       fp16
Device pipeline per 512-row j-quad (j on partitions, local i free):
    q < 8 : slot <- DMA(adjP[q]);  slot += s1_bcast   (DVE TT 2x)
    q >= 8: slot += DMA-accum(adjP[q])                (SWDGE compute DMA;
            slot already holds adj[q-8]*40 + s1, delta restores adj[q]+s1)
    y    = prelu(slot + (s2_j-40), 0.2) -> work       (ACT, bias per tile;
                                         some tiles on DVE TS+STT instead)
    bits = int16(y*A + B)                (DVE TS 4x; Schraudolph exp:
                                          bitcast fp16 ~= exp(y))
    psum[c,i] += hs[j,c].T @ bits.as_fp16[j,i]        (hs as PE weights)
    out = elu(num/den) after a PE transpose of the [67, rl] accumulator.
Masked entries: x ~= s1+s2-40 -> y ~= 0.2x -> exp(y) ~ e^-8, negligible
vs row mass; no explicit mask multiply or -inf needed.
"""

import os
import sys

for _p in ("/opt/trn_rl_repo",):
    if os.path.isdir(_p) and _p not in sys.path:
        sys.path.append(_p)

import numpy as np

import concourse.bass as bass
import concourse.bacc as bacc
import concourse.mybir as mybir
import concourse.tile as tile
from concourse import bass_utils

N, D, F = 8192, 256, 64
NCORES = 8
RL = N // NCORES
BIG = 40.0
ALPHA = 0.2
# Schraudolph fp16 exp: bitcast_f16(int16(A*y + B)) ~= e^y, A = 2^10/ln2,
# B = 15360 - 61 (61 centers the mantissa-linearization error).
SCH_A = 1477.3196
SCH_B = 15299.0
RING = 8                  # adj slot ring; quads q>=RING are delta-encoded
DVE_LEAKY = 0             # j-tiles whose leaky runs on DVE instead of ACT
USE_DMA_ACCUM = False     # DRAM->SBUF accum during DMA (SWDGE compute)

f32 = mybir.dt.float32
fp16 = mybir.dt.float16
i16 = mybir.dt.int16
Alu = mybir.AluOpType
Act = mybir.ActivationFunctionType

LAST_RESULTS = None
_CACHE = {}


def _kernel_body(tc, out_d, featP_d, featTl_d, adjP_d, W_d, a_d, idn_d, n=N, rl=RL):
    nc = tc.nc
    nit = rl // 128           # local i-tiles
    njt = n // 128            # global j-tiles
    nk = D // 128             # d contraction tiles
    QT = 4                    # j-tiles per chain quad
    NQ = njt // QT
    HC = F + 3                # hs cols: h(64) | s1 | s2 | ones
    NXC = 4                   # X^T streamed in chunks along j
    jxc = n // NXC

    with (
        tc.tile_pool(name="sbP", bufs=1) as sbP,
        tc.tile_pool(name="sbS", bufs=2) as sbS,
        tc.tile_pool(name="sbA", bufs=RING) as sbA,
        tc.tile_pool(name="sbL", bufs=3) as sbL,
        tc.tile_pool(name="sbE", bufs=4) as sbE,
        tc.tile_pool(name="pp", bufs=6, space="PSUM") as pp,
        tc.tile_pool(name="pacc", bufs=1, space="PSUM") as pacc,
    ):
        aq = [
            sbA.tile([128, QT, rl], fp16, tag="aq", name=f"aq{q}") for q in range(NQ)
        ]

        # ---- plain adj loads for the first RING quads: no deps, start t=0 -
        for q0 in range(min(RING, NQ)):
            nc.gpsimd.dma_start(aq[q0][:], adjP_d[q0])

        # ---- prologue feeds on the sync (HWDGE) queue ---------------------
        arow = sbP.tile([1, 2 * F], f32)
        nc.sync.dma_start(arow[:], a_d.rearrange("f o -> o f"))
        wsb = sbP.tile([128, nk, F], f32)
        nc.sync.dma_start(wsb[:], W_d.rearrange("(k p) f -> p k f", p=128))
        idn = sbP.tile([128, 128], f32)
        nc.sync.dma_start(idn[:], idn_d)
        xTl = sbP.tile([128, nk, rl], fp16)
        nc.sync.dma_start(xTl[:], featTl_d.rearrange("(k p) i -> p k i", p=128))

        xTf = [
            sbS.tile([128, nk, jxc], fp16, tag="xTf", name=f"xTf{i}")
            for i in range(NXC)
        ]
        for i in range(NXC):
            nc.sync.dma_start(xTf[i][:], featP_d[i])

        # ---- constants / rhs16 --------------------------------------------
        onesf = sbP.tile([1, 128], f32)
        nc.vector.memset(onesf[:], 1.0)
        ab = sbP.tile([128, 2 * F], f32)
        psab = pp.tile([128, 2 * F], f32, tag="big", name="psab")
        nc.tensor.matmul(psab[:], onesf[:], arow[:])
        nc.vector.tensor_copy(ab[:], psab[:])
        wa = sbP.tile([128, nk, 2], f32)
        scr = sbP.tile([128, F], f32)
        for k in range(nk):
            nc.vector.scalar_tensor_tensor(
                scr[:], wsb[:, k, :], 1.0, ab[:, :F], Alu.mult, Alu.mult,
                accum_out=wa[:, k, 0:1],
            )
            nc.vector.scalar_tensor_tensor(
                scr[:], wsb[:, k, :], 1.0, ab[:, F:], Alu.mult, Alu.mult,
                accum_out=wa[:, k, 1:2],
            )
        rhs16 = sbP.tile([128, nk, F + 2], fp16)
        for k in range(nk):
            nc.vector.tensor_copy(rhs16[:, k, :F], wsb[:, k, :])
            nc.vector.tensor_copy(rhs16[:, k, F : F + 2], wa[:, k, :])

        # ---- fast s1: [1, rl] row via wa1-as-weights matmul ---------------
        ones1 = sbP.tile([1, 128], fp16)
        nc.vector.memset(ones1[:], 1.0)
        s1row = sbP.tile([1, rl], fp16)
        for h in range(0, rl, 512):
            ps1 = pp.tile([1, 512], f32, tag="big", name=f"ps1_{h}")
            for k in range(nk):
                nc.tensor.matmul(
                    ps1[:], rhs16[:, k, F : F + 1], xTl[:, k, h : h + 512],
                    start=(k == 0), stop=(k == nk - 1),
                )
            nc.vector.tensor_copy(s1row[:, h : h + 512], ps1[:])
        s1b4 = sbP.tile([128, QT, rl], fp16)
        for h in range(0, rl, 512):
            psb = pp.tile([128, 512], f32, tag="big", name=f"psb{h}")
            nc.tensor.matmul(psb[:], ones1[:], s1row[:, h : h + 512])
            nc.vector.tensor_copy(s1b4[:, 0, h : h + 512], psb[:])
        for tt in range(1, QT):
            nc.vector.tensor_copy(s1b4[:, tt, :], s1b4[:, 0, :])

        # ---- hs blocks interleaved with attention quads -------------------
        hs_all = sbP.tile([128, njt, HC], fp16)
        nc.vector.memset(hs_all[:, :, F + 2 : F + 3], 1.0)
        # s2 - 40, per-partition scalars per j-tile (fp32)
        s2mB = sbP.tile([128, njt], f32)

        def hs_block(g):
            psh = pp.tile([128, QT, F + 2], f32, tag="big", name=f"psh{g}")
            for tt in range(QT):
                t = g * QT + tt
                cx, ct = divmod(t, jxc // 128)
                for k in range(nk):
                    nc.tensor.matmul(
                        psh[:, tt, :], xTf[cx][:, k, ct * 128 : (ct + 1) * 128],
                        rhs16[:, k, :],
                        start=(k == 0), stop=(k == nk - 1),
                    )
            nc.scalar.copy(hs_all[:, g * QT : (g + 1) * QT, : F + 2], psh[:])
            nc.vector.tensor_scalar(
                s2mB[:, g * QT : (g + 1) * QT],
                hs_all[:, g * QT : (g + 1) * QT, F + 1],
                -BIG, None, Alu.add,
            )

        lt = [
            sbL.tile([128, QT, rl], i16, tag="lt", name=f"lt{q}") for q in range(NQ)
        ]
        # hs-as-weights accumulator: [HC, rl] += hs[:,t,:].T @ P[:, i]
        pso = pacc.tile([HC, rl], f32, name="pso")

        HS_AHEAD = 2
        for g in range(min(HS_AHEAD, NQ)):
            hs_block(g)

        ndve = 0
        for q in range(NQ):
            if q + HS_AHEAD < NQ:
                hs_block(q + HS_AHEAD)
            w = aq[q]
            # x1 = adj*40 + s1 (in place)
            wflat = w[:].rearrange("p t i -> p (t i)")
            nc.vector.tensor_tensor(
                wflat, wflat, s1b4[:].rearrange("p t i -> p (t i)"), Alu.add
            )
            for tt in range(QT):
                t = q * QT + tt
                if tt == QT - 1 and ndve < DVE_LEAKY:
                    ndve += 1
                    nc.vector.tensor_scalar(
                        w[:, tt, :], w[:, tt, :], s2mB[:, t : t + 1], None, Alu.add
                    )
                    nc.vector.scalar_tensor_tensor(
                        w[:, tt, :], w[:, tt, :], ALPHA, w[:, tt, :],
                        Alu.mult, Alu.max,
                    )
                else:
                    nc.scalar.activation(
                        w[:, tt, :], w[:, tt, :], Act.Prelu,
                        bias=s2mB[:, t : t + 1], alpha=ALPHA,
                    )
            ltf = lt[q][:].rearrange("p t i -> p (t i)")
            nc.vector.tensor_scalar(ltf, wflat, SCH_A, SCH_B, Alu.mult, Alu.add)
            w16 = lt[q][:].bitcast(fp16)
            for tt in range(QT):
                t = q * QT + tt
                for hh in range(0, rl, 512):
                    nc.tensor.matmul(
                        pso[:, hh : hh + 512], hs_all[:, t, :],
                        w16[:, tt, hh : hh + 512],
                        start=(t == 0), stop=(t == njt - 1),
                    )
            # DMA for quad q+RING lands on this quad's slot
            if q + RING < NQ:
                nc.gpsimd.dma_start(aq[q + RING][:], adjP_d[q + RING])

        # ---- epilogue: PE-transpose pso chunks, divide, elu ---------------
        psof = sbE.tile([HC, rl], f32, tag="psof", bufs=1)
        nc.vector.tensor_copy(psof[:], pso[:])
        for it in range(nit):
            pst = pp.tile([128, HC], f32, tag="big", name=f"pst{it}")
            nc.tensor.transpose(
                pst[:, :], psof[:, it * 128 : (it + 1) * 128], idn[:HC, :HC]
            )
            rcp = sbE.tile([128, 1], f32, tag="rcp")
            nc.vector.reciprocal(rcp[:], pst[:, F + 2 : F + 3])
            o = sbE.tile([128, F], f32, tag="o")
            nc.vector.tensor_scalar_mul(o[:], pst[:, :F], rcp[:])
            q2 = sbE.tile([128, F], f32, tag="q2")
            nc.vector.tensor_scalar_min(q2[:], o[:], 0.0)
            e = sbE.tile([128, F], f32, tag="e")
            nc.scalar.activation(e[:], q2[:], Act.Exp)
            r = sbE.tile([128, F], f32, tag="r")
            nc.vector.tensor_scalar_max(r[:], o[:], 0.0)
            fin = sbE.tile([128, F], f32, tag="fin")
            nc.vector.scalar_tensor_tensor(
                fin[:], e[:], -1.0, r[:], Alu.add, Alu.add
            )
            nc.sync.dma_start(out_d[it * 128 : (it + 1) * 128, :], fin[:])


def _build(n=N, rl=RL, ncores=NCORES):
    key = (n, rl, ncores)
    if key in _CACHE:
        return _CACHE[key]
    nc = bacc.Bacc(
        "TRN2", target_bir_lowering=False, debug=False, num_devices=ncores
    )
    njt = n // 128
    NQ = njt // 4
    jxc = n // 4
    nk = D // 128
    featP = nc.dram_tensor("featP", [4, 128, nk, jxc], fp16, kind="ExternalInput").ap()
    featTl = nc.dram_tensor("featTl", [D, rl], fp16, kind="ExternalInput").ap()
    adjP = nc.dram_tensor("adjP", [NQ, 128, 4, rl], fp16, kind="ExternalInput").ap()
    W = nc.dram_tensor("W", [D, F], f32, kind="ExternalInput").ap()
    a = nc.dram_tensor("a", [2 * F, 1], f32, kind="ExternalInput").ap()
    idn = nc.dram_tensor("idn", [128, 128], f32, kind="ExternalInput").ap()
    out = nc.dram_tensor("out", [rl, F], f32, kind="ExternalOutput").ap()
    with tile.TileContext(nc) as tc:
        _kernel_body(tc, out, featP, featTl, adjP, W, a, idn, n=n, rl=rl)
    nc.compile()
    _CACHE[key] = nc
    return nc


def kernel(features, adj, W, a):
    global LAST_RESULTS
    features = np.ascontiguousarray(features, dtype=np.float32)
    adj = np.ascontiguousarray(adj, dtype=np.int32)
    W = np.ascontiguousarray(W, dtype=np.float32)
    a = np.ascontiguousarray(a, dtype=np.float32)

    n = adj.shape[0]
    rl = n // NCORES
    njt = n // 128
    NQ = njt // 4
    nk = D // 128
    jxc = n // 4
    nc = _build(n=n, rl=rl, ncores=NCORES)
    # featP[c, p, k, j] = features.T[(k*128+p), c*jxc + j]
    fT16 = features.T.astype(np.float16)                    # [D, n]
    featP = np.ascontiguousarray(
        fT16.reshape(nk, 128, 4, jxc).transpose(2, 1, 0, 3)
    )
    idn = np.eye(128, dtype=np.float32)
    in_maps = []
    for c in range(NCORES):
        adjT40 = adj[c * rl : (c + 1) * rl].T.astype(np.float16) * np.float16(BIG)
        # adjP[Q, p, t, i] = adjT40[Q*512 + t*128 + p, i]; delta-encode Q>=RING
        adjP = adjT40.reshape(NQ, 4, 128, rl).transpose(0, 2, 1, 3).copy()
        if USE_DMA_ACCUM:
            adjP[RING:] = adjP[RING:] - adjP[: NQ - RING]
        in_maps.append(
            {
                "featP": featP,
                "featTl": np.ascontiguousarray(
                    features[c * rl : (c + 1) * rl].T.astype(np.float16)
                ),
                "adjP": np.ascontiguousarray(adjP),
                "W": W,
                "a": a,
                "idn": idn,
            }
        )
    res = bass_utils.run_bass_kernel_spmd(nc, in_maps, core_ids=list(range(NCORES)))
    LAST_RESULTS = res
    return np.concatenate([res.results[c]["out"] for c in range(NCORES)], axis=0)
